# revision 1
# baseline (speedup 1.0000x reference)
"""Trainium2 Bass kernel for nn_Activity_Detection: 3-modality bidirectional
LSTM activity head.

Sharding (8 NeuronCores): 4 batch shards (128 rows) x 2 LSTM directions.
Cores 0-3 run the forward LSTMs, cores 4-7 the reverse LSTMs on host
time-reversed features; one SPMD program. Each core computes, per modality,
projT = (x @ W.T + b).T on the PE (audio's 128-wide projection is folded into
its gate weights on the host), then a 20-step LSTM recurrence in bf16 with
fp32 PSUM accumulation, and finally a partial output
(h_a*h_r*h_c) @ W_out_half.T in fp32. The host sums fwd+rev partials + b_out.
"""

import numpy as np
import ml_dtypes

import concourse.bass as bass
import concourse.bacc as bacc
import concourse.tile as tile
import concourse.mybir as mybir
from concourse.masks import make_identity
from concourse import bass_utils

BF16 = mybir.dt.bfloat16
F32 = mybir.dt.float32
AF = mybir.ActivationFunctionType

B, T = 512, 20
RES, C3D, AUD, P, H, NCLS = 2048, 4096, 128, 1024, 1024, 200
BS = 128          # batch rows per core
G4 = 4 * H        # 4096 gate dim
NKH = H // 128    # 8 h chunks
MODS = ("audio", "resnet", "c3d")
DIMS = {"audio": AUD, "resnet": RES, "c3d": C3D}

TRACE = False            # set by test harness for profiling
LAST_RESULTS = None      # BassKernelResults of the last run (for profiling)


def build_program(has_gate_bias: bool):
    nc = bacc.Bacc("TRN2", target_bir_lowering=False, debug=False, num_devices=1)

    x_d = {m: nc.dram_tensor(f"x_{m}", [T, BS, DIMS[m]], BF16, kind="ExternalInput").ap()
           for m in MODS}
    wt_d = {m: nc.dram_tensor(f"wt_{m}", [DIMS[m], P], BF16, kind="ExternalInput").ap()
            for m in ("resnet", "c3d")}
    bp_d = {m: nc.dram_tensor(f"bp_{m}", [P], F32, kind="ExternalInput").ap()
            for m in ("resnet", "c3d")}
    kd = {"audio": AUD + H, "resnet": P + H, "c3d": P + H}
    ww_d = {m: nc.dram_tensor(f"ww_{m}", [kd[m], G4], BF16, kind="ExternalInput").ap()
            for m in MODS}
    gb_d = {m: nc.dram_tensor(f"gb_{m}", [G4], F32, kind="ExternalInput").ap()
            for m in MODS}
    wout_d = nc.dram_tensor("wout", [H, NCLS], F32, kind="ExternalInput").ap()
    out_d = nc.dram_tensor("out_partial", [BS, NCLS], F32, kind="ExternalOutput").ap()
    pjt_d = {m: nc.dram_tensor(f"pjt_{m}", [T, NKH, 128, BS], BF16, kind="Internal").ap()
             for m in ("resnet", "c3d")}

    from contextlib import ExitStack
    with tile.TileContext(nc) as tc, ExitStack() as stack:
        const = stack.enter_context(tc.tile_pool(name="const", bufs=1))
        psum = stack.enter_context(tc.tile_pool(name="psum", bufs=6, space="PSUM"))
        tpsum = stack.enter_context(tc.tile_pool(name="tpsum", bufs=2, space="PSUM"))

        ident_bf = const.tile([128, 128], BF16)
        make_identity(nc, ident_bf[:])
        ident_f32 = const.tile([128, 128], F32)
        make_identity(nc, ident_f32[:])
        fused_acc = const.tile([128, H], F32)

        xt_a = const.tile([128, T, 128], BF16)

        # ---------------- phase A: projections to DRAM ----------------
        RCH = 512  # rows per proj chunk (4 time steps)
        with (
            tc.tile_pool(name="wtp", bufs=1) as wtp,
            tc.tile_pool(name="xtp", bufs=3) as xtp,
            tc.tile_pool(name="evp", bufs=4) as evp,
        ):
            for m in ("c3d", "resnet"):
                if m == "resnet":
                    # audio xT transposes ride the c3d proj tail instead of
                    # delaying the first wt loads
                    for t in range(T):
                        nc.sync.dma_start_transpose(xt_a[:, t, :], x_d["audio"][t])
                dk = DIMS[m] // 128
                bp = const.tile([128, NKH], F32, tag=f"bp_{m}")
                nc.sync.dma_start(bp[:], bp_d[m].rearrange("(mo p) -> p mo", p=128))
                wt = wtp.tile([128, dk, P], BF16, tag="wt")
                wtr = wt_d[m].rearrange("(ko p) n -> p ko n", p=128)
                for k in range(dk):
                    nc.sync.dma_start(wt[:, k], wtr[:, k])
                for r in range(0, T * BS // RCH):
                    xt = xtp.tile([128, dk, RCH], BF16, tag="xt")
                    for tt in range(RCH // BS):
                        t = (r * RCH) // BS + tt
                        nc.sync.dma_start_transpose(
                            xt[:, :, tt * BS:(tt + 1) * BS], x_d[m][t])
                    for half in (0, 1):
                        pp = [psum.tile([128, 512], F32, tag="ps",
                                        name=f"pj_{m}_{r}_{half}_{mm}")
                              for mm in range(4)]
                        for k in range(dk):
                            for mm in range(4):
                                nc.tensor.matmul(
                                    pp[mm][:, 0:RCH],
                                    wt[:, k, (half * 4 + mm) * 128:
                                             (half * 4 + mm + 1) * 128],
                                    xt[:, k, :],
                                    start=(k == 0), stop=(k == dk - 1))
                        for mm in range(4):
                            mo = half * 4 + mm
                            ev = evp.tile([128, RCH], BF16, tag="ev")
                            nc.scalar.activation(
                                ev[:], pp[mm][:, 0:RCH], AF.Identity,
                                bias=bp[:, mo:mo + 1])
                            for tt in range(RCH // BS):
                                t = (r * RCH) // BS + tt
                                nc.sync.dma_start(
                                    pjt_d[m][t, mo],
                                    ev[:, tt * BS:(tt + 1) * BS])

        # ---------------- phase B: recurrences ----------------
        work = stack.enter_context(tc.tile_pool(name="work", bufs=2))
        state = stack.enter_context(tc.tile_pool(name="state", bufs=1))
        wwp = stack.enter_context(tc.tile_pool(name="wwp", bufs=1))
        pjs = stack.enter_context(tc.tile_pool(name="pjs", bufs=3))

        def recurrence(m, st_x_fn, n_kx):
            n_k = n_kx + NKH
            ww = wwp.tile([128, n_k, G4], BF16, tag="ww")
            wwr = ww_d[m].rearrange("(ko p) n -> p ko n", p=128)
            for k in range(n_k):
                nc.sync.dma_start(ww[:, k], wwr[:, k])
            if has_gate_bias:
                gb_sb = wwp.tile([128, G4], F32, tag="gb")
                nc.sync.dma_start(gb_sb[:], gb_d[m][None, :].to_broadcast([128, G4]))
            hT = state.tile([128, NKH, 128], BF16, tag="hT")
            c_st = state.tile([128, H], F32, tag="c_st")
            h_bf = state.tile([128, H], BF16, tag="h_bf")

            for t in range(T):
                def st(k):
                    return st_x_fn(t, k) if k < n_kx else hT[:, k - n_kx, :]

                ks = list(range(n_k)) if t > 0 else list(range(n_kx))
                G = [psum.tile([128, 512], F32, tag="ps", name=f"g_{m}_{t}_{n}")
                     for n in range(8)]
                for half in (0, 1):
                    for k in ks:
                        for n in range(half * 4, half * 4 + 4):
                            nc.tensor.matmul(
                                G[n][:], st(k), ww[:, k, n * 512:(n + 1) * 512],
                                start=(k == ks[0]), stop=(k == ks[-1]))

                # gate n-chunks: i: G[0:2], f: G[2:4], g: G[4:6], o: G[6:8]
                for j in (0, 1):
                    def gate_in(idx):
                        src = G[idx][:]
                        if has_gate_bias:
                            gs = work.tile([128, 512], F32, tag="gsb")
                            nc.vector.tensor_add(
                                gs[:], src, gb_sb[:, idx * 512:(idx + 1) * 512])
                            src = gs[:]
                        return src

                    sl = slice(j * 512, (j + 1) * 512)
                    sig_f = work.tile([128, 512], F32, tag="sig_f")
                    nc.scalar.activation(sig_f[:], gate_in(2 + j), AF.Sigmoid)
                    if t > 0:
                        nc.vector.tensor_mul(c_st[:, sl], sig_f[:], c_st[:, sl])
                    sig_i = work.tile([128, 512], F32, tag="sig_i")
                    nc.scalar.activation(sig_i[:], gate_in(0 + j), AF.Sigmoid)
                    tanh_g = work.tile([128, 512], F32, tag="tanh_g")
                    nc.scalar.activation(tanh_g[:], gate_in(4 + j), AF.Tanh)
                    if t > 0:
                        tmp2 = work.tile([128, 512], F32, tag="tmp2")
                        nc.vector.tensor_mul(tmp2[:], sig_i[:], tanh_g[:])
                        nc.vector.tensor_add(c_st[:, sl], c_st[:, sl], tmp2[:])
                    else:
                        nc.vector.tensor_mul(c_st[:, sl], sig_i[:], tanh_g[:])
                    tc_t = work.tile([128, 512], F32, tag="tc_t")
                    nc.scalar.activation(tc_t[:], c_st[:, sl], AF.Tanh)
                    sig_o = work.tile([128, 512], F32, tag="sig_o")
                    nc.scalar.activation(sig_o[:], gate_in(6 + j), AF.Sigmoid)
                    if t < T - 1:
                        nc.vector.tensor_mul(h_bf[:, sl], sig_o[:], tc_t[:])
                    else:
                        if m == "audio":
                            nc.vector.tensor_mul(fused_acc[:, sl], sig_o[:], tc_t[:])
                        else:
                            hf = work.tile([128, 512], F32, tag="hf")
                            nc.vector.tensor_mul(hf[:], sig_o[:], tc_t[:])
                            nc.vector.tensor_mul(
                                fused_acc[:, sl], fused_acc[:, sl], hf[:])
                if t < T - 1:
                    for k in range(NKH):
                        tp = tpsum.tile([128, 512], F32, tag="tp",
                                        name=f"tp_{m}_{t}_{k}")
                        tpv = tp[:, 0:128].bitcast(BF16)[:, 0:128]
                        nc.tensor.transpose(
                            tpv, h_bf[:, k * 128:(k + 1) * 128], ident_bf[:])
                        nc.vector.tensor_copy(hT[:, k, :], tpv)

        recurrence("audio", lambda t, k: xt_a[:, t, :], 1)

        for m in ("resnet", "c3d"):
            pjt_tiles = {}

            def stream_pjt(t, k, m=m, pjt_tiles=pjt_tiles):
                if t not in pjt_tiles:
                    pt = pjs.tile([128, NKH, BS], BF16, tag="pjt")
                    nc.sync.dma_start(
                        pt[:], pjt_d[m][t].rearrange("mo p b -> p mo b"))
                    pjt_tiles.clear()
                    pjt_tiles[t] = pt
                return pjt_tiles[t][:, k, :]

            recurrence(m, stream_pjt, NKH)

        # ---------------- final ----------------
        with tc.tile_pool(name="fin", bufs=1) as fin:
            wo = fin.tile([128, NKH, NCLS], F32)
            nc.sync.dma_start(wo[:], wout_d.rearrange("(ko p) n -> p ko n", p=128))
            ops = psum.tile([128, 512], F32, tag="ps", name="out_ps")
            for k in range(NKH):
                tp = tpsum.tile([128, 512], F32, tag="tp", name=f"ft_{k}")
                nc.tensor.transpose(
                    tp[:, 0:128], fused_acc[:, k * 128:(k + 1) * 128], ident_f32[:])
                ft = work.tile([128, 128], F32, tag="ft")
                nc.vector.tensor_copy(ft[:], tp[:, 0:128])
                nc.tensor.matmul(ops[:, 0:NCLS], ft[:], wo[:, k, :],
                                 start=(k == 0), stop=(k == NKH - 1))
            osb = work.tile([128, NCLS], F32, tag="osb")
            nc.vector.tensor_copy(osb[:], ops[:, 0:NCLS])
            nc.sync.dma_start(out_d[:], osb[:])

    nc.compile()
    return nc


def _bf16(a):
    return np.ascontiguousarray(a).astype(ml_dtypes.bfloat16)


def host_prep(inputs):
    f = np.float32
    xs = {"audio": inputs["audio_features"], "resnet": inputs["resnet_features"],
          "c3d": inputs["c3d_features"]}
    xt = {m: np.swapaxes(np.asarray(v, f), 0, 1) for m, v in xs.items()}

    wt = {"resnet": _bf16(np.asarray(inputs["W_resnet"], f).T),
          "c3d": _bf16(np.asarray(inputs["W_c3d"], f).T)}
    bp = {"resnet": np.asarray(inputs["b_resnet"], f),
          "c3d": np.asarray(inputs["b_c3d"], f)}

    dirs = {}
    has_gate_bias = False
    for d in ("fwd", "rev"):
        ww = {}
        gb = {}
        for m in MODS:
            wih = np.asarray(inputs[f"{m}_{d}_Wih"], f)
            whh = np.asarray(inputs[f"{m}_{d}_Whh"], f)
            bih = np.asarray(inputs[f"{m}_{d}_bih"], f)
            bhh = np.asarray(inputs[f"{m}_{d}_bhh"], f)
            if m == "audio":
                wa = np.asarray(inputs["W_audio"], f)
                wcomb = wih @ wa                        # (4H, AUD)
                ww[m] = _bf16(np.concatenate([wcomb.T, whh.T], axis=0))
                gb[m] = (wih @ np.asarray(inputs["b_audio"], f) + bih + bhh).astype(f)
            else:
                ww[m] = _bf16(np.concatenate([wih.T, whh.T], axis=0))
                gb[m] = (bih + bhh).astype(f)
            if np.any(gb[m] != 0):
                has_gate_bias = True
        wout_half = (np.asarray(inputs["W_out"], f)[:, :H].T if d == "fwd"
                     else np.asarray(inputs["W_out"], f)[:, H:].T)
        dirs[d] = {"ww": ww, "gb": gb, "wout": np.ascontiguousarray(wout_half)}

    in_maps = []
    for core in range(8):
        d = "fwd" if core < 4 else "rev"
        s = core % 4
        rows = slice(s * BS, (s + 1) * BS)
        im = {}
        for m in MODS:
            xm = xt[m][:, rows]
            if d == "rev":
                xm = xm[::-1]
            im[f"x_{m}"] = _bf16(xm)
            im[f"ww_{m}"] = dirs[d]["ww"][m]
            im[f"gb_{m}"] = dirs[d]["gb"][m]
        for m in ("resnet", "c3d"):
            im[f"wt_{m}"] = wt[m]
            im[f"bp_{m}"] = bp[m]
        im["wout"] = dirs[d]["wout"]
        in_maps.append(im)
    return in_maps, has_gate_bias


def assemble(results, inputs):
    out = np.zeros((B, NCLS), np.float32)
    for s in range(4):
        rows = slice(s * BS, (s + 1) * BS)
        out[rows] = results[s]["out_partial"] + results[4 + s]["out_partial"]
    out += np.asarray(inputs["b_out"], np.float32)[None, :]
    return out


def kernel(**inputs):
    global LAST_RESULTS
    in_maps, has_gate_bias = host_prep(inputs)
    nc = build_program(has_gate_bias)
    res = bass_utils.run_bass_kernel_spmd(
        nc, in_maps, core_ids=list(range(8)), trace=TRACE)
    LAST_RESULTS = res
    return assemble(res.results, inputs)



# revision 6
# speedup vs baseline: 1.0642x; 1.0642x over previous
"""Trainium2 Bass kernel for nn_Activity_Detection: 3-modality bidirectional
LSTM activity head.

Sharding (8 NeuronCores): 4 batch shards (128 rows) x 2 LSTM directions.
Cores 0-3 run the forward LSTMs, cores 4-7 the reverse LSTMs on host
time-reversed features; one SPMD program.

Schedule (v2): phases arranged so the PE never waits on elementwise tails or
on the slow transposing DMAs of the feature loads:
  phase 1: resnet projection (solo; ww_audio + audio xT stream in behind it)
  phase 2: audio recurrence interleaved with the c3d projection (the
           feed-forward projection matmuls fill audio's elementwise bubbles;
           projection front-loaded so the c3d weight buffers free early and
           the resnet gate weights can prefetch under the audio tail)
  phase 3: resnet recurrence (c3d gate-weight head chunk prefetches in slack)
  phase 4: c3d recurrence + output head
Gate columns are host-permuted to [i0 f0 g0 o0 | i1 f1 g1 o1] (512-wide
chunks) so each 2048-column half's four PSUM banks finish consecutively and
the elementwise work for half 0 overlaps the matmuls of half 1. Within a
step, the hT-independent x-part matmuls are emitted before the h transposes
of the previous step so the PE always has ready work while the previous
step's elementwise tail drains.
"""

import numpy as np
import ml_dtypes

import concourse.bass as bass
import concourse.bacc as bacc
import concourse.tile as tile
import concourse.mybir as mybir
from concourse.masks import make_identity
from concourse import bass_utils

BF16 = mybir.dt.bfloat16
F32 = mybir.dt.float32
AF = mybir.ActivationFunctionType

B, T = 512, 20
RES, C3D, AUD, P, H, NCLS = 2048, 4096, 128, 1024, 1024, 200
BS = 128          # batch rows per core
G4 = 4 * H        # 4096 gate dim (columns host-permuted)
NKH = H // 128    # 8 h chunks
MODS = ("audio", "resnet", "c3d")
DIMS = {"audio": AUD, "resnet": RES, "c3d": C3D}

TRACE = False            # set by test harness for profiling
LAST_RESULTS = None      # BassKernelResults of the last run (for profiling)


def build_program(has_gate_bias: bool):
    nc = bacc.Bacc("TRN2", target_bir_lowering=False, debug=False, num_devices=1)

    x_d = {m: nc.dram_tensor(f"x_{m}", [T, BS, DIMS[m]], BF16, kind="ExternalInput").ap()
           for m in MODS}
    wt_d = {m: nc.dram_tensor(f"wt_{m}", [DIMS[m], P], BF16, kind="ExternalInput").ap()
            for m in ("resnet", "c3d")}
    bp_d = {m: nc.dram_tensor(f"bp_{m}", [P], F32, kind="ExternalInput").ap()
            for m in ("resnet", "c3d")}
    kd = {"audio": AUD + H, "resnet": P + H, "c3d": P + H}
    ww_d = {m: nc.dram_tensor(f"ww_{m}", [kd[m], G4], BF16, kind="ExternalInput").ap()
            for m in MODS}
    gb_d = {m: nc.dram_tensor(f"gb_{m}", [G4], F32, kind="ExternalInput").ap()
            for m in MODS}
    wout_d = nc.dram_tensor("wout", [H, NCLS], F32, kind="ExternalInput").ap()
    out_d = nc.dram_tensor("out_partial", [BS, NCLS], F32, kind="ExternalOutput").ap()
    pjt_d = {m: nc.dram_tensor(f"pjt_{m}", [T, NKH, 128, BS], BF16, kind="Internal").ap()
             for m in ("resnet", "c3d")}

    from contextlib import ExitStack
    with tile.TileContext(nc) as tc, ExitStack() as stack:
        const = stack.enter_context(tc.tile_pool(name="const", bufs=1))
        state = stack.enter_context(tc.tile_pool(name="state", bufs=1))
        work = stack.enter_context(tc.tile_pool(name="work", bufs=2))
        tpsum = stack.enter_context(tc.tile_pool(name="tpsum", bufs=2, space="PSUM"))
        pjs = stack.enter_context(tc.tile_pool(name="pjs", bufs=3))

        ident_bf = const.tile([128, 128], BF16)
        make_identity(nc, ident_bf[:])
        ident_f32 = const.tile([128, 128], F32)
        make_identity(nc, ident_f32[:])
        fused_acc = const.tile([128, H], F32)
        xt_a = const.tile([128, T, 128], BF16)

        gb_sb = {}
        if has_gate_bias:
            for m in MODS:
                gb_sb[m] = const.tile([128, G4], F32, tag=f"gb_{m}")
                nc.sync.dma_start(gb_sb[m][:], gb_d[m][None, :].to_broadcast([128, G4]))

        # ---------- shared recurrence-step emission ----------
        def ew_half(m, t, j, G, c_st, h_bf):
            """Elementwise for gate-column half j given its 4 PSUM banks
            G = [i, f, g, o]."""
            sl = slice(j * 512, (j + 1) * 512)

            def gin(b):
                src = G[b][:]
                if has_gate_bias:
                    gs = work.tile([128, 512], F32, tag="gs")
                    nc.vector.tensor_add(
                        gs[:], src, gb_sb[m][:, (j * 4 + b) * 512:(j * 4 + b + 1) * 512])
                    src = gs[:]
                return src

            sf = work.tile([128, 512], F32, tag="sf")
            nc.scalar.activation(sf[:], gin(1), AF.Sigmoid)
            if t > 0:
                nc.vector.tensor_mul(c_st[:, sl], sf[:], c_st[:, sl])
            si = work.tile([128, 512], F32, tag="si")
            nc.scalar.activation(si[:], gin(0), AF.Sigmoid)
            tg = work.tile([128, 512], F32, tag="tg")
            nc.scalar.activation(tg[:], gin(2), AF.Tanh)
            if t > 0:
                tmp = work.tile([128, 512], F32, tag="sf")
                nc.vector.tensor_mul(tmp[:], si[:], tg[:])
                nc.vector.tensor_add(c_st[:, sl], c_st[:, sl], tmp[:])
            else:
                nc.vector.tensor_mul(c_st[:, sl], si[:], tg[:])
            tc_t = work.tile([128, 512], F32, tag="tg")
            nc.scalar.activation(tc_t[:], c_st[:, sl], AF.Tanh)
            so = work.tile([128, 512], F32, tag="sf")
            nc.scalar.activation(so[:], gin(3), AF.Sigmoid)
            if t < T - 1:
                nc.vector.tensor_mul(h_bf[:, sl], so[:], tc_t[:])
            else:
                if m == "audio":
                    nc.vector.tensor_mul(fused_acc[:, sl], so[:], tc_t[:])
                else:
                    hf = work.tile([128, 512], F32, tag="si")
                    nc.vector.tensor_mul(hf[:], so[:], tc_t[:])
                    nc.vector.tensor_mul(fused_acc[:, sl], fused_acc[:, sl], hf[:])

        def emit_recurrence(m, gp, ww_at, n_kx, st_x_fn, filler=None):
            """One full 20-step recurrence for modality m.

            gp: PSUM tile pool for the gate banks.
            ww_at(k): AP of the [128, G4] weight row-chunk k (k < n_kx: x-part,
                      k >= n_kx: h-part).
            st_x_fn(t, k): stationary [128,128] x chunk for step t.
            filler(t): optional callback emitting independent PE work.
            """
            n_k = n_kx + NKH
            hT = state.tile([128, NKH, 128], BF16, tag="hT")
            c_st = state.tile([128, H], F32, tag="c_st")
            h_bf = state.tile([128, H], BF16, tag="h_bf")

            for t in range(T):
                if filler is not None:
                    filler(t)
                G0 = [gp.tile([128, 512], F32, tag="g", name=f"g_{m}_{t}_0_{b}")
                      for b in range(4)]
                # half 0, x-part (independent of hT(t-1)), k-outer b-inner
                for k in range(n_kx):
                    for b in range(4):
                        nc.tensor.matmul(
                            G0[b][:], st_x_fn(t, k), ww_at(k)[:, b * 512:(b + 1) * 512],
                            start=(k == 0), stop=(t == 0 and k == n_kx - 1))
                if t > 0:
                    # h transposes of the previous step (wait on ew(t-1))
                    for k in range(NKH):
                        tp = tpsum.tile([128, 512], F32, tag="tp",
                                        name=f"tp_{m}_{t}_{k}")
                        tpv = tp[:, 0:128].bitcast(BF16)[:, 0:128]
                        nc.tensor.transpose(
                            tpv, h_bf[:, k * 128:(k + 1) * 128], ident_bf[:])
                        nc.vector.tensor_copy(hT[:, k - 0, :], tpv)
                    # half 0, h-part
                    for k in range(n_kx, n_k):
                        for b in range(4):
                            nc.tensor.matmul(
                                G0[b][:], hT[:, k - n_kx, :],
                                ww_at(k)[:, b * 512:(b + 1) * 512],
                                start=False, stop=(k == n_k - 1))
                ew_half(m, t, 0, G0, c_st, h_bf)
                # half 1: bank-outer k-inner (staggers first-writes past the
                # half-0 elementwise reads of the rotating PSUM banks)
                G1 = [gp.tile([128, 512], F32, tag="g", name=f"g_{m}_{t}_1_{b}")
                      for b in range(4)]
                last = (n_kx if t == 0 else n_k) - 1
                for b in range(4):
                    for k in range(last + 1):
                        nc.tensor.matmul(
                            G1[b][:],
                            (st_x_fn(t, k) if k < n_kx else hT[:, k - n_kx, :]),
                            ww_at(k)[:, (4 + b) * 512:(4 + b + 1) * 512],
                            start=(k == 0), stop=(k == last))
                ew_half(m, t, 1, G1, c_st, h_bf)

        # pjt streaming for resnet/c3d recurrences (eager prefetch of t=0)
        def make_pjt_stream(m):
            tiles = {}

            def prefetch(t):
                if t < T and t not in tiles:
                    pt = pjs.tile([128, NKH, BS], BF16, tag="pjt")
                    nc.sync.dma_start(
                        pt[:], pjt_d[m][t].rearrange("mo p b -> p mo b"))
                    tiles[t] = pt

            prefetch(0)

            def st_x(t, k):
                prefetch(t)
                if k == 0:
                    prefetch(t + 1)
                for tt in [tt for tt in tiles if tt < t - 1]:
                    del tiles[tt]
                return tiles[t][:, k, :]

            return st_x

        # ---------------- phase 1: resnet projection ----------------
        with tc.tile_pool(name="wwp_a", bufs=1) as wwp_a:
            with nc.named_scope("p1_resproj"):
                with (
                    tc.tile_pool(name="wtp_r", bufs=1) as wtp_r,
                    tc.tile_pool(name="xtp_r", bufs=2) as xtp_r,
                    tc.tile_pool(name="evp_r", bufs=3) as evp_r,
                    tc.tile_pool(name="projp_r", bufs=3, space="PSUM") as projp_r,
                ):
                    bp_r = const.tile([128, NKH], F32, tag="bp_resnet")
                    nc.sync.dma_start(
                        bp_r[:], bp_d["resnet"].rearrange("(mo p) -> p mo", p=128))
                    dk_r = RES // 128
                    wt_r = wtp_r.tile([128, dk_r, P], BF16, tag="wt_r")
                    wtr = wt_d["resnet"].rearrange("(ko p) n -> p ko n", p=128)
                    for k in range(dk_r):
                        nc.sync.dma_start(wt_r[:, k], wtr[:, k])
                    RCH = 512
                    for r in range(T * BS // RCH):
                        xt = xtp_r.tile([128, dk_r, RCH], BF16, tag="xt")
                        for tt in range(RCH // BS):
                            t = (r * RCH) // BS + tt
                            nc.sync.dma_start_transpose(
                                xt[:, :, tt * BS:(tt + 1) * BS], x_d["resnet"][t])
                        for mo in range(NKH):
                            pp = projp_r.tile([128, 512], F32, tag="pp",
                                              name=f"pjr_{r}_{mo}")
                            for k in range(dk_r):
                                nc.tensor.matmul(
                                    pp[:], wt_r[:, k, mo * 128:(mo + 1) * 128],
                                    xt[:, k, :], start=(k == 0), stop=(k == dk_r - 1))
                            ev = evp_r.tile([128, RCH], BF16, tag="ev")
                            nc.scalar.activation(ev[:], pp[:], AF.Identity,
                                                 bias=bp_r[:, mo:mo + 1])
                            for tt in range(RCH // BS):
                                t = (r * RCH) // BS + tt
                                nc.sync.dma_start(
                                    pjt_d["resnet"][t, mo],
                                    ev[:, tt * BS:(tt + 1) * BS])
                        if r == 1:
                            # phase-2 loads ride the phase-1 tail
                            wwa = wwp_a.tile([128, 1 + NKH, G4], BF16, tag="ww_a")
                            wwar = ww_d["audio"].rearrange("(ko p) n -> p ko n", p=128)
                            for k in range(1 + NKH):
                                nc.sync.dma_start(wwa[:, k], wwar[:, k])
                            for t in range(T):
                                nc.sync.dma_start_transpose(
                                    xt_a[:, t, :], x_d["audio"][t])

            # -------- phase 2: audio recurrence || c3d projection --------
            with nc.named_scope("p2_audio_c3dproj"):
                with (
                    tc.tile_pool(name="wtp_c", bufs=1) as wtp_c,
                    tc.tile_pool(name="xtp_c", bufs=2) as xtp_c,
                    tc.tile_pool(name="evp_c", bufs=2) as evp_c,
                    tc.tile_pool(name="projp_c", bufs=2, space="PSUM") as projp_c,
                    tc.tile_pool(name="gp2", bufs=4, space="PSUM") as gp2,
                ):
                    bp_c = const.tile([128, NKH], F32, tag="bp_c3d")
                    nc.sync.dma_start(
                        bp_c[:], bp_d["c3d"].rearrange("(mo p) -> p mo", p=128))
                    dk_c = C3D // 128
                    wt_c = wtp_c.tile([128, dk_c, P], BF16, tag="wt_c")
                    wtc = wt_d["c3d"].rearrange("(ko p) n -> p ko n", p=128)
                    for k in range(dk_c):
                        nc.sync.dma_start(wt_c[:, k], wtc[:, k])

                    RC2 = 256  # c3d proj rows per slab (2 time steps)
                    n_half = 2 * (T * BS // RC2)   # 20 half-slabs (4 mo each)
                    cur_xt = {}

                    def c3d_half_slab(h):
                        r, half = h // 2, h % 2
                        if half == 0:
                            xt = xtp_c.tile([128, dk_c, RC2], BF16, tag="xt")
                            for tt in range(RC2 // BS):
                                t = (r * RC2) // BS + tt
                                nc.sync.dma_start_transpose(
                                    xt[:, :, tt * BS:(tt + 1) * BS], x_d["c3d"][t])
                            cur_xt["xt"] = xt
                        xt = cur_xt["xt"]
                        for mo in range(half * 4, half * 4 + 4):
                            pp = projp_c.tile([128, 512], F32, tag="pp",
                                              name=f"pjc_{r}_{mo}")
                            for k in range(dk_c):
                                nc.tensor.matmul(
                                    pp[:, 0:RC2], wt_c[:, k, mo * 128:(mo + 1) * 128],
                                    xt[:, k, :], start=(k == 0), stop=(k == dk_c - 1))
                            ev = evp_c.tile([128, RC2], BF16, tag="ev")
                            nc.scalar.activation(ev[:], pp[:, 0:RC2], AF.Identity,
                                                 bias=bp_c[:, mo:mo + 1])
                            for tt in range(RC2 // BS):
                                t = (r * RC2) // BS + tt
                                nc.sync.dma_start(
                                    pjt_d["c3d"][t, mo],
                                    ev[:, tt * BS:(tt + 1) * BS])

                    # front-loaded: none at t=0 (wt_c still streaming in),
                    # then 2 half-slabs/step, tapering off by t=13
                    hctr = [0]

                    def filler(t):
                        want = 0 if t == 0 else (2 if t < 9 else (1 if t < 13 else 0))
                        for _ in range(want):
                            if hctr[0] < n_half:
                                c3d_half_slab(hctr[0])
                                hctr[0] += 1

                    emit_recurrence(
                        "audio", gp2,
                        ww_at=lambda k: wwa[:, k],
                        n_kx=1,
                        st_x_fn=lambda t, k: xt_a[:, t, :],
                        filler=filler)
                    while hctr[0] < n_half:
                        c3d_half_slab(hctr[0])
                        hctr[0] += 1

        # wwp_a closed: audio gate weights freed.
        with tc.tile_pool(name="wwcx", bufs=1) as wwcx:
            # ---------------- phase 3: resnet recurrence ----------------
            with nc.named_scope("p3_resrec"):
                with (
                    tc.tile_pool(name="wwp_r", bufs=1) as wwp_r,
                    tc.tile_pool(name="gp3", bufs=6, space="PSUM") as gp3,
                ):
                    st_res = make_pjt_stream("resnet")
                    ww_r = wwp_r.tile([128, P // 128 + NKH, G4], BF16, tag="ww_r")
                    wwrr = ww_d["resnet"].rearrange("(ko p) n -> p ko n", p=128)
                    for k in range(P // 128 + NKH):
                        nc.sync.dma_start(ww_r[:, k], wwrr[:, k])
                    # prefetch first 4 c3d gate-weight chunks into phase-3 slack
                    ww_c_x4 = wwcx.tile([128, 4, G4], BF16, tag="ww_c_x4")
                    wwcr = ww_d["c3d"].rearrange("(ko p) n -> p ko n", p=128)
                    for k in range(4):
                        nc.sync.dma_start(ww_c_x4[:, k], wwcr[:, k])

                    emit_recurrence(
                        "resnet", gp3,
                        ww_at=lambda k: ww_r[:, k],
                        n_kx=P // 128,
                        st_x_fn=st_res)

            # ---------------- phase 4: c3d recurrence + head ----------------
            with nc.named_scope("p4_c3drec"):
                with (
                    tc.tile_pool(name="wwp_c", bufs=1) as wwp_c,
                    tc.tile_pool(name="gp4", bufs=6, space="PSUM") as gp4,
                ):
                    st_c3d = make_pjt_stream("c3d")
                    n_rest = P // 128 + NKH - 4
                    ww_c_rest = wwp_c.tile([128, n_rest, G4], BF16, tag="ww_c_rest")
                    for k in range(n_rest):
                        nc.sync.dma_start(ww_c_rest[:, k], wwcr[:, k + 4])

                    emit_recurrence(
                        "c3d", gp4,
                        ww_at=lambda k: ww_c_x4[:, k] if k < 4 else ww_c_rest[:, k - 4],
                        n_kx=P // 128,
                        st_x_fn=st_c3d)

                    # ---------------- output head ----------------
                    with tc.tile_pool(name="fin", bufs=1) as fin:
                        wo = fin.tile([128, NKH, NCLS], F32)
                        nc.sync.dma_start(
                            wo[:], wout_d.rearrange("(ko p) n -> p ko n", p=128))
                        ops = gp4.tile([128, 512], F32, tag="g", name="out_ps")
                        for k in range(NKH):
                            tp = tpsum.tile([128, 512], F32, tag="tp",
                                            name=f"ft_{k}")
                            nc.tensor.transpose(
                                tp[:, 0:128], fused_acc[:, k * 128:(k + 1) * 128],
                                ident_f32[:])
                            ft = work.tile([128, 128], F32, tag="ft")
                            nc.vector.tensor_copy(ft[:], tp[:, 0:128])
                            nc.tensor.matmul(ops[:, 0:NCLS], ft[:], wo[:, k, :],
                                             start=(k == 0), stop=(k == NKH - 1))
                        osb = work.tile([128, NCLS], F32, tag="osb")
                        nc.vector.tensor_copy(osb[:], ops[:, 0:NCLS])
                        nc.sync.dma_start(out_d[:], osb[:])

    nc.compile()
    return nc


def _bf16(a):
    return np.ascontiguousarray(a).astype(ml_dtypes.bfloat16)


# gate-column permutation: [i0 f0 g0 o0 i1 f1 g1 o1] (512-wide chunks)
_GPERM = np.concatenate(
    [np.arange(b * H + j * 512, b * H + j * 512 + 512)
     for j in (0, 1) for b in range(4)])


def host_prep(inputs):
    f = np.float32
    xs = {"audio": inputs["audio_features"], "resnet": inputs["resnet_features"],
          "c3d": inputs["c3d_features"]}
    xt = {m: np.swapaxes(np.asarray(v, f), 0, 1) for m, v in xs.items()}

    wt = {"resnet": _bf16(np.asarray(inputs["W_resnet"], f).T),
          "c3d": _bf16(np.asarray(inputs["W_c3d"], f).T)}
    bp = {"resnet": np.asarray(inputs["b_resnet"], f),
          "c3d": np.asarray(inputs["b_c3d"], f)}

    dirs = {}
    has_gate_bias = False
    for d in ("fwd", "rev"):
        ww = {}
        gb = {}
        for m in MODS:
            wih = np.asarray(inputs[f"{m}_{d}_Wih"], f)
            whh = np.asarray(inputs[f"{m}_{d}_Whh"], f)
            bih = np.asarray(inputs[f"{m}_{d}_bih"], f)
            bhh = np.asarray(inputs[f"{m}_{d}_bhh"], f)
            if m == "audio":
                wa = np.asarray(inputs["W_audio"], f)
                wcomb = wih @ wa                        # (4H, AUD)
                wwm = np.concatenate([wcomb.T, whh.T], axis=0)
                gbm = (wih @ np.asarray(inputs["b_audio"], f) + bih + bhh).astype(f)
            else:
                wwm = np.concatenate([wih.T, whh.T], axis=0)
                gbm = (bih + bhh).astype(f)
            ww[m] = _bf16(wwm[:, _GPERM])
            gb[m] = np.ascontiguousarray(gbm[_GPERM])
            if np.any(gb[m] != 0):
                has_gate_bias = True
        wout_half = (np.asarray(inputs["W_out"], f)[:, :H].T if d == "fwd"
                     else np.asarray(inputs["W_out"], f)[:, H:].T)
        dirs[d] = {"ww": ww, "gb": gb, "wout": np.ascontiguousarray(wout_half)}

    in_maps = []
    for core in range(8):
        d = "fwd" if core < 4 else "rev"
        s = core % 4
        rows = slice(s * BS, (s + 1) * BS)
        im = {}
        for m in MODS:
            xm = xt[m][:, rows]
            if d == "rev":
                xm = xm[::-1]
            im[f"x_{m}"] = _bf16(xm)
            im[f"ww_{m}"] = dirs[d]["ww"][m]
            im[f"gb_{m}"] = dirs[d]["gb"][m]
        for m in ("resnet", "c3d"):
            im[f"wt_{m}"] = wt[m]
            im[f"bp_{m}"] = bp[m]
        im["wout"] = dirs[d]["wout"]
        in_maps.append(im)
    return in_maps, has_gate_bias


def assemble(results, inputs):
    out = np.zeros((B, NCLS), np.float32)
    for s in range(4):
        rows = slice(s * BS, (s + 1) * BS)
        out[rows] = results[s]["out_partial"] + results[4 + s]["out_partial"]
    out += np.asarray(inputs["b_out"], np.float32)[None, :]
    return out


def kernel(**inputs):
    global LAST_RESULTS
    in_maps, has_gate_bias = host_prep(inputs)
    nc = build_program(has_gate_bias)
    res = bass_utils.run_bass_kernel_spmd(
        nc, in_maps, core_ids=list(range(8)), trace=TRACE)
    LAST_RESULTS = res
    return assemble(res.results, inputs)


# revision 26
# speedup vs baseline: 1.1563x; 1.0865x over previous
"""Trainium2 Bass kernel for nn_Activity_Detection: 3-modality bidirectional
LSTM activity head.

Sharding (8 NeuronCores): 4 batch shards (128 rows) x 2 LSTM directions.
Cores 0-3 run the forward LSTMs, cores 4-7 the reverse LSTMs on host
time-reversed features; one SPMD program.

Projection dedup (v3): the fwd core s and rev core s+4 need the *same*
resnet/c3d projections (the rev core's features are the host-time-reversed
copy, so proj_rev[t] == proj_fwd[19-t]). Each core computes only its own
t=0..9 half, the pair AllGathers the halves (replica groups [s, s+4]), and
steps t>=10 read both gathered slots blended with a per-core 0/1 mask input
(fwd wants slot1, rev slot0) — keeping the program SPMD-uniform. This halves
both the projection matmuls and the transposing feature DMAs.

Schedule: phases arranged so the PE never waits on elementwise tails or
on the slow transposing DMAs of the feature loads:
  phase 1: resnet projection half (solo; ww_audio + audio xT behind it),
           then the resnet pjt AllGather
  phase 2: audio recurrence interleaved with the c3d projection half (the
           feed-forward projection matmuls fill audio's elementwise bubbles;
           the c3d weight buffers free early so the resnet gate weights can
           prefetch under the audio tail), then the c3d pjt AllGather
  phase 3: resnet recurrence (c3d gate-weight head chunk prefetches in slack)
  phase 4: c3d recurrence + output head
Gate columns are host-permuted to [i0 f0 g0 o0 | i1 f1 g1 o1] (512-wide
chunks) so each 2048-column half's four PSUM banks finish consecutively and
the elementwise work for half 0 overlaps the matmuls of half 1. Within a
step, the hT-independent x-part matmuls are emitted before the h transposes
of the previous step so the PE always has ready work while the previous
step's elementwise tail drains.
"""

import numpy as np
import ml_dtypes

import concourse.bass as bass
import concourse.bacc as bacc
import concourse.tile as tile
import concourse.mybir as mybir
from concourse.masks import make_identity
from concourse import bass_utils

BF16 = mybir.dt.bfloat16
F32 = mybir.dt.float32
AF = mybir.ActivationFunctionType

B, T = 512, 20
TH = T // 2       # projection half computed locally per core
RES, C3D, AUD, P, H, NCLS = 2048, 4096, 128, 1024, 1024, 200
BS = 128          # batch rows per core
G4 = 4 * H        # 4096 gate dim (columns host-permuted)
NKH = H // 128    # 8 h chunks
PAIR_GROUPS = [[0, 4], [1, 5], [2, 6], [3, 7]]
MODS = ("audio", "resnet", "c3d")
DIMS = {"audio": AUD, "resnet": RES, "c3d": C3D}

TRACE = False            # set by test harness for profiling
LAST_RESULTS = None      # BassKernelResults of the last run (for profiling)


def build_program(has_gate_bias: bool):
    nc = bacc.Bacc("TRN2", target_bir_lowering=False, debug=False, num_devices=8)

    x_d = {m: nc.dram_tensor(f"x_{m}", [T, BS, DIMS[m]], BF16, kind="ExternalInput").ap()
           for m in MODS}
    wt_d = {m: nc.dram_tensor(f"wt_{m}", [DIMS[m], P], BF16, kind="ExternalInput").ap()
            for m in ("resnet", "c3d")}
    bp_d = {m: nc.dram_tensor(f"bp_{m}", [P], F32, kind="ExternalInput").ap()
            for m in ("resnet", "c3d")}
    kd = {"audio": AUD + H, "resnet": P + H, "c3d": P + H}
    ww_d = {m: nc.dram_tensor(f"ww_{m}", [kd[m], G4], BF16, kind="ExternalInput").ap()
            for m in MODS}
    gb_d = {m: nc.dram_tensor(f"gb_{m}", [G4], F32, kind="ExternalInput").ap()
            for m in MODS}
    wout_d = nc.dram_tensor("wout", [H, NCLS], F32, kind="ExternalInput").ap()
    dmask_d = nc.dram_tensor("dmask", [128, 2], F32, kind="ExternalInput").ap()
    out_d = nc.dram_tensor("out_partial", [BS, NCLS], F32, kind="ExternalOutput").ap()
    # own projection half (t < TH) and the pair-gathered both-halves buffer
    pjt_d = {m: nc.dram_tensor(f"pjt_{m}", [TH, NKH, 128, BS], BF16, kind="Internal").ap()
             for m in ("resnet", "c3d")}
    pjg_d = {m: nc.dram_tensor(f"pjg_{m}", [2, TH, NKH, 128, BS], BF16, kind="Internal").ap()
             for m in ("resnet", "c3d")}

    from contextlib import ExitStack
    with tile.TileContext(nc) as tc, ExitStack() as stack:
        const = stack.enter_context(tc.tile_pool(name="const", bufs=1))
        state = stack.enter_context(tc.tile_pool(name="state", bufs=1))
        work = stack.enter_context(tc.tile_pool(name="work", bufs=2))
        tpsum = stack.enter_context(tc.tile_pool(name="tpsum", bufs=2, space="PSUM"))

        ident_bf = const.tile([128, 128], BF16)
        make_identity(nc, ident_bf[:])
        ident_f32 = const.tile([128, 128], F32)
        make_identity(nc, ident_f32[:])
        fused_acc = const.tile([128, H], F32)
        dmask = const.tile([128, 2], F32)
        nc.sync.dma_start(dmask[:], dmask_d[:])

        gb_sb = {}
        if has_gate_bias:
            for m in MODS:
                gb_sb[m] = const.tile([128, G4], F32, tag=f"gb_{m}")
                nc.sync.dma_start(gb_sb[m][:], gb_d[m][None, :].to_broadcast([128, G4]))

        # ---------- shared recurrence-step emission ----------
        def ew_half(m, t, j, G, c_st, h_bf):
            """Elementwise for gate-column half j given its 4 PSUM banks
            G = [i, f, g, o]."""
            sl = slice(j * 512, (j + 1) * 512)

            def gin(b):
                src = G[b][:]
                if has_gate_bias:
                    gs = work.tile([128, 512], F32, tag="gs")
                    nc.vector.tensor_add(
                        gs[:], src, gb_sb[m][:, (j * 4 + b) * 512:(j * 4 + b + 1) * 512])
                    src = gs[:]
                return src

            sf = work.tile([128, 512], F32, tag="sf")
            nc.scalar.activation(sf[:], gin(1), AF.Sigmoid)
            if t > 0:
                nc.vector.tensor_mul(c_st[:, sl], sf[:], c_st[:, sl])
            si = work.tile([128, 512], F32, tag="si")
            nc.scalar.activation(si[:], gin(0), AF.Sigmoid)
            tg = work.tile([128, 512], F32, tag="tg")
            nc.scalar.activation(tg[:], gin(2), AF.Tanh)
            if t > 0:
                tmp = work.tile([128, 512], F32, tag="sf")
                nc.vector.tensor_mul(tmp[:], si[:], tg[:])
                nc.vector.tensor_add(c_st[:, sl], c_st[:, sl], tmp[:])
            else:
                nc.vector.tensor_mul(c_st[:, sl], si[:], tg[:])
            tc_t = work.tile([128, 512], F32, tag="tg")
            nc.scalar.activation(tc_t[:], c_st[:, sl], AF.Tanh)
            so = work.tile([128, 512], F32, tag="sf")
            nc.scalar.activation(so[:], gin(3), AF.Sigmoid)
            if t < T - 1:
                nc.vector.tensor_mul(h_bf[:, sl], so[:], tc_t[:])
            else:
                if m == "audio":
                    nc.vector.tensor_mul(fused_acc[:, sl], so[:], tc_t[:])
                else:
                    hf = work.tile([128, 512], F32, tag="si")
                    nc.vector.tensor_mul(hf[:], so[:], tc_t[:])
                    nc.vector.tensor_mul(fused_acc[:, sl], fused_acc[:, sl], hf[:])

        def emit_recurrence(m, gp, ww_at, n_kx, st_x_fn, filler=None):
            """One full 20-step recurrence for modality m.

            gp: PSUM tile pool for the gate banks.
            ww_at(k): AP of the [128, G4] weight row-chunk k (k < n_kx: x-part,
                      k >= n_kx: h-part).
            st_x_fn(t, k): stationary [128,128] x chunk for step t.
            filler(t): optional callback emitting independent PE work.
            """
            n_k = n_kx + NKH
            hT = state.tile([128, NKH, 128], BF16, tag="hT")
            c_st = state.tile([128, H], F32, tag="c_st")
            h_bf = state.tile([128, H], BF16, tag="h_bf")

            for t in range(T):
                if filler is not None:
                    filler(t)
                G0 = [gp.tile([128, 512], F32, tag="g", name=f"g_{m}_{t}_0_{b}")
                      for b in range(4)]
                # half 0, x-part (independent of hT(t-1)), k-outer b-inner
                for k in range(n_kx):
                    for b in range(4):
                        nc.tensor.matmul(
                            G0[b][:], st_x_fn(t, k), ww_at(k)[:, b * 512:(b + 1) * 512],
                            start=(k == 0), stop=(t == 0 and k == n_kx - 1))
                if t > 0:
                    # h transposes of the previous step (wait on ew(t-1))
                    for k in range(NKH):
                        tp = tpsum.tile([128, 512], F32, tag="tp",
                                        name=f"tp_{m}_{t}_{k}")
                        tpv = tp[:, 0:128].bitcast(BF16)[:, 0:128]
                        nc.tensor.transpose(
                            tpv, h_bf[:, k * 128:(k + 1) * 128], ident_bf[:])
                        nc.vector.tensor_copy(hT[:, k - 0, :], tpv)
                    # half 0, h-part
                    for k in range(n_kx, n_k):
                        for b in range(4):
                            nc.tensor.matmul(
                                G0[b][:], hT[:, k - n_kx, :],
                                ww_at(k)[:, b * 512:(b + 1) * 512],
                                start=False, stop=(k == n_k - 1))
                ew_half(m, t, 0, G0, c_st, h_bf)
                # half 1: bank-outer k-inner (staggers first-writes past the
                # half-0 elementwise reads of the rotating PSUM banks)
                G1 = [gp.tile([128, 512], F32, tag="g", name=f"g_{m}_{t}_1_{b}")
                      for b in range(4)]
                last = (n_kx if t == 0 else n_k) - 1
                for b in range(4):
                    for k in range(last + 1):
                        nc.tensor.matmul(
                            G1[b][:],
                            (st_x_fn(t, k) if k < n_kx else hT[:, k - n_kx, :]),
                            ww_at(k)[:, (4 + b) * 512:(4 + b + 1) * 512],
                            start=(k == 0), stop=(k == last))
                ew_half(m, t, 1, G1, c_st, h_bf)

        # pjt streaming for resnet/c3d recurrences (eager prefetch of t=0).
        # t < TH: own local half. t >= TH: blend of the two gathered slots
        # (slot1 for fwd cores, slot0 for rev cores, selected by dmask).
        def make_pjt_stream(m, pjs):
            tiles = {}

            def prefetch(t):
                if t >= T or t in tiles:
                    return
                if t < TH:
                    pt = pjs.tile([128, NKH, BS], BF16, tag="pjt")
                    nc.sync.dma_start(
                        pt[:], pjt_d[m][t].rearrange("mo p b -> p mo b"))
                    tiles[t] = pt
                else:
                    pa = pjs.tile([128, NKH, BS], BF16, tag="pjA", bufs=2)
                    pb = pjs.tile([128, NKH, BS], BF16, tag="pjB", bufs=2)
                    px = pjs.tile([128, NKH, BS], BF16, tag="pjX", bufs=2)
                    nc.sync.dma_start(
                        pa[:], pjg_d[m][0, T - 1 - t].rearrange("mo p b -> p mo b"))
                    nc.sync.dma_start(
                        pb[:], pjg_d[m][1, T - 1 - t].rearrange("mo p b -> p mo b"))
                    # px = pb*m + pa*(1-m)
                    nc.vector.tensor_scalar_mul(px[:], pb[:], dmask[:, 0:1])
                    nc.vector.scalar_tensor_tensor(
                        px[:], pa[:], dmask[:, 1:2], px[:],
                        mybir.AluOpType.mult, mybir.AluOpType.add)
                    tiles[t] = px

            prefetch(0)

            def st_x(t, k):
                prefetch(t)
                if k == 0:
                    prefetch(t + 1)
                for tt in [tt for tt in tiles if tt < t - 1]:
                    del tiles[tt]
                return tiles[t][:, k, :]

            return st_x

        # ---------------- phase 1: resnet projection (own half) ----------------
        with tc.tile_pool(name="wwp_a", bufs=1) as wwp_a:
            xt_a = wwp_a.tile([128, T, 128], BF16, tag="xt_a")
            with nc.named_scope("p1_resproj"):
                with (
                    tc.tile_pool(name="wtp_r", bufs=1) as wtp_r,
                    tc.tile_pool(name="xtp_r", bufs=3) as xtp_r,
                    tc.tile_pool(name="evp_r", bufs=3) as evp_r,
                    tc.tile_pool(name="projp_r", bufs=3, space="PSUM") as projp_r,
                ):
                    bp_r = const.tile([128, NKH], F32, tag="bp_resnet")
                    nc.sync.dma_start(
                        bp_r[:], bp_d["resnet"].rearrange("(mo p) -> p mo", p=128))
                    dk_r = RES // 128
                    wt_r = wtp_r.tile([128, dk_r, P], BF16, tag="wt_r")
                    wtr = wt_d["resnet"].rearrange("(ko p) n -> p ko n", p=128)
                    for k in range(dk_r):
                        nc.sync.dma_start(wt_r[:, k], wtr[:, k])
                    wwa = wwp_a.tile([128, 1 + NKH, G4], BF16, tag="ww_a")
                    wwar = ww_d["audio"].rearrange("(ko p) n -> p ko n", p=128)
                    RCH = 256
                    for r in range(TH * BS // RCH):
                        xt = xtp_r.tile([128, dk_r, RCH], BF16, tag="xt")
                        for tt in range(RCH // BS):
                            t = (r * RCH) // BS + tt
                            nc.sync.dma_start_transpose(
                                xt[:, :, tt * BS:(tt + 1) * BS], x_d["resnet"][t])
                        for mo in range(NKH):
                            pp = projp_r.tile([128, 512], F32, tag="pp",
                                              name=f"pjr_{r}_{mo}")
                            for k in range(dk_r):
                                nc.tensor.matmul(
                                    pp[:, 0:RCH], wt_r[:, k, mo * 128:(mo + 1) * 128],
                                    xt[:, k, :], start=(k == 0), stop=(k == dk_r - 1))
                            ev = evp_r.tile([128, RCH], BF16, tag="ev")
                            nc.scalar.activation(ev[:], pp[:, 0:RCH], AF.Identity,
                                                 bias=bp_r[:, mo:mo + 1])
                            for tt in range(RCH // BS):
                                t = (r * RCH) // BS + tt
                                nc.sync.dma_start(
                                    pjt_d["resnet"][t, mo],
                                    ev[:, tt * BS:(tt + 1) * BS])
                        # phase-2 loads ride the phase-1 tail, spread so they
                        # never starve the xt feed
                        if r in (1, 2, 3):
                            for k in range(3 * (r - 1), 3 * r):
                                nc.sync.dma_start(wwa[:, k], wwar[:, k])
                        if r == 4:
                            for t in range(T):
                                nc.sync.dma_start_transpose(
                                    xt_a[:, t, :], x_d["audio"][t])
                # pair-exchange the resnet projection halves
                nc.gpsimd.collective_compute(
                    "AllGather", mybir.AluOpType.bypass,
                    replica_groups=PAIR_GROUPS,
                    ins=[pjt_d["resnet"].opt()],
                    outs=[pjg_d["resnet"].opt()])

            # -------- phase 2: audio recurrence || c3d projection --------
            with nc.named_scope("p2_audio_c3dproj"):
                with (
                    tc.tile_pool(name="wtp_c", bufs=1) as wtp_c,
                    tc.tile_pool(name="xtp_c", bufs=2) as xtp_c,
                    tc.tile_pool(name="evp_c", bufs=2) as evp_c,
                    tc.tile_pool(name="projp_c", bufs=2, space="PSUM") as projp_c,
                    tc.tile_pool(name="gp2", bufs=4, space="PSUM") as gp2,
                ):
                    bp_c = const.tile([128, NKH], F32, tag="bp_c3d")
                    nc.sync.dma_start(
                        bp_c[:], bp_d["c3d"].rearrange("(mo p) -> p mo", p=128))
                    dk_c = C3D // 128
                    wt_c = wtp_c.tile([128, dk_c, P], BF16, tag="wt_c")
                    wtc = wt_d["c3d"].rearrange("(ko p) n -> p ko n", p=128)
                    for k in range(dk_c):
                        nc.sync.dma_start(wt_c[:, k], wtc[:, k])

                    RC2 = 256  # c3d proj rows per slab (2 time steps)
                    n_half = 2 * (TH * BS // RC2)  # 10 half-slabs (4 mo each)
                    cur_xt = {}

                    def c3d_half_slab(h):
                        r, half = h // 2, h % 2
                        if half == 0:
                            xt = xtp_c.tile([128, dk_c, RC2], BF16, tag="xt")
                            for tt in range(RC2 // BS):
                                t = (r * RC2) // BS + tt
                                nc.sync.dma_start_transpose(
                                    xt[:, :, tt * BS:(tt + 1) * BS], x_d["c3d"][t])
                            cur_xt["xt"] = xt
                        xt = cur_xt["xt"]
                        for mo in range(half * 4, half * 4 + 4):
                            pp = projp_c.tile([128, 512], F32, tag="pp",
                                              name=f"pjc_{r}_{mo}")
                            for k in range(dk_c):
                                nc.tensor.matmul(
                                    pp[:, 0:RC2], wt_c[:, k, mo * 128:(mo + 1) * 128],
                                    xt[:, k, :], start=(k == 0), stop=(k == dk_c - 1))
                            ev = evp_c.tile([128, RC2], BF16, tag="ev")
                            nc.scalar.activation(ev[:], pp[:, 0:RC2], AF.Identity,
                                                 bias=bp_c[:, mo:mo + 1])
                            for tt in range(RC2 // BS):
                                t = (r * RC2) // BS + tt
                                nc.sync.dma_start(
                                    pjt_d["c3d"][t, mo],
                                    ev[:, tt * BS:(tt + 1) * BS])

                    # one half-slab per step from t=2 (wt_c streams in during
                    # t=0..1); done by t=11 so the c3d buffers free early
                    hctr = [0]

                    def filler(t):
                        if 2 <= t and hctr[0] < n_half:
                            c3d_half_slab(hctr[0])
                            hctr[0] += 1

                    emit_recurrence(
                        "audio", gp2,
                        ww_at=lambda k: wwa[:, k],
                        n_kx=1,
                        st_x_fn=lambda t, k: xt_a[:, t, :],
                        filler=filler)
                    while hctr[0] < n_half:
                        c3d_half_slab(hctr[0])
                        hctr[0] += 1
                # pair-exchange the c3d projection halves
                nc.gpsimd.collective_compute(
                    "AllGather", mybir.AluOpType.bypass,
                    replica_groups=PAIR_GROUPS,
                    ins=[pjt_d["c3d"].opt()],
                    outs=[pjg_d["c3d"].opt()])

        # wwp_a closed: audio gate weights freed.
        with (
            tc.tile_pool(name="wwcx", bufs=1) as wwcx,
            tc.tile_pool(name="pjs", bufs=3) as pjs,
        ):
            # ---------------- phase 3: resnet recurrence ----------------
            with nc.named_scope("p3_resrec"):
                with (
                    tc.tile_pool(name="wwp_r", bufs=1) as wwp_r,
                    tc.tile_pool(name="gp3", bufs=6, space="PSUM") as gp3,
                ):
                    st_res = make_pjt_stream("resnet", pjs)
                    ww_r = wwp_r.tile([128, P // 128 + NKH, G4], BF16, tag="ww_r")
                    wwrr = ww_d["resnet"].rearrange("(ko p) n -> p ko n", p=128)
                    for k in range(P // 128 + NKH):
                        nc.sync.dma_start(ww_r[:, k], wwrr[:, k])
                    # prefetch first 3 c3d gate-weight chunks into phase-3 slack
                    NCX = 3
                    ww_c_x4 = wwcx.tile([128, NCX, G4], BF16, tag="ww_c_x4")
                    wwcr = ww_d["c3d"].rearrange("(ko p) n -> p ko n", p=128)
                    for k in range(NCX):
                        nc.sync.dma_start(ww_c_x4[:, k], wwcr[:, k])

                    emit_recurrence(
                        "resnet", gp3,
                        ww_at=lambda k: ww_r[:, k],
                        n_kx=P // 128,
                        st_x_fn=st_res)

            # ---------------- phase 4: c3d recurrence + head ----------------
            with nc.named_scope("p4_c3drec"):
                with (
                    tc.tile_pool(name="wwp_c", bufs=1) as wwp_c,
                    tc.tile_pool(name="gp4", bufs=6, space="PSUM") as gp4,
                ):
                    st_c3d = make_pjt_stream("c3d", pjs)
                    n_rest = P // 128 + NKH - NCX
                    ww_c_rest = wwp_c.tile([128, n_rest, G4], BF16, tag="ww_c_rest")
                    for k in range(n_rest):
                        nc.sync.dma_start(ww_c_rest[:, k], wwcr[:, k + NCX])

                    emit_recurrence(
                        "c3d", gp4,
                        ww_at=lambda k: (ww_c_x4[:, k] if k < NCX
                                         else ww_c_rest[:, k - NCX]),
                        n_kx=P // 128,
                        st_x_fn=st_c3d)

                    # ---------------- output head ----------------
                    with tc.tile_pool(name="fin", bufs=1) as fin:
                        wo = fin.tile([128, NKH, NCLS], F32)
                        nc.sync.dma_start(
                            wo[:], wout_d.rearrange("(ko p) n -> p ko n", p=128))
                        ops = gp4.tile([128, 512], F32, tag="g", name="out_ps")
                        for k in range(NKH):
                            tp = tpsum.tile([128, 512], F32, tag="tp",
                                            name=f"ft_{k}")
                            nc.tensor.transpose(
                                tp[:, 0:128], fused_acc[:, k * 128:(k + 1) * 128],
                                ident_f32[:])
                            ft = fin.tile([128, 128], F32, tag="ft", bufs=2)
                            nc.vector.tensor_copy(ft[:], tp[:, 0:128])
                            nc.tensor.matmul(ops[:, 0:NCLS], ft[:], wo[:, k, :],
                                             start=(k == 0), stop=(k == NKH - 1))
                        osb = fin.tile([128, NCLS], F32, tag="osb")
                        nc.vector.tensor_copy(osb[:], ops[:, 0:NCLS])
                        nc.sync.dma_start(out_d[:], osb[:])

    nc.compile()
    return nc


def _bf16(a):
    return np.ascontiguousarray(a).astype(ml_dtypes.bfloat16)


# gate-column permutation: [i0 f0 g0 o0 i1 f1 g1 o1] (512-wide chunks)
_GPERM = np.concatenate(
    [np.arange(b * H + j * 512, b * H + j * 512 + 512)
     for j in (0, 1) for b in range(4)])


def host_prep(inputs):
    f = np.float32
    xs = {"audio": inputs["audio_features"], "resnet": inputs["resnet_features"],
          "c3d": inputs["c3d_features"]}
    xt = {m: np.swapaxes(np.asarray(v, f), 0, 1) for m, v in xs.items()}

    wt = {"resnet": _bf16(np.asarray(inputs["W_resnet"], f).T),
          "c3d": _bf16(np.asarray(inputs["W_c3d"], f).T)}
    bp = {"resnet": np.asarray(inputs["b_resnet"], f),
          "c3d": np.asarray(inputs["b_c3d"], f)}

    dirs = {}
    has_gate_bias = False
    for d in ("fwd", "rev"):
        ww = {}
        gb = {}
        for m in MODS:
            wih = np.asarray(inputs[f"{m}_{d}_Wih"], f)
            whh = np.asarray(inputs[f"{m}_{d}_Whh"], f)
            bih = np.asarray(inputs[f"{m}_{d}_bih"], f)
            bhh = np.asarray(inputs[f"{m}_{d}_bhh"], f)
            if m == "audio":
                wa = np.asarray(inputs["W_audio"], f)
                wcomb = wih @ wa                        # (4H, AUD)
                wwm = np.concatenate([wcomb.T, whh.T], axis=0)
                gbm = (wih @ np.asarray(inputs["b_audio"], f) + bih + bhh).astype(f)
            else:
                wwm = np.concatenate([wih.T, whh.T], axis=0)
                gbm = (bih + bhh).astype(f)
            ww[m] = _bf16(wwm[:, _GPERM])
            gb[m] = np.ascontiguousarray(gbm[_GPERM])
            if np.any(gb[m] != 0):
                has_gate_bias = True
        wout_half = (np.asarray(inputs["W_out"], f)[:, :H].T if d == "fwd"
                     else np.asarray(inputs["W_out"], f)[:, H:].T)
        dirs[d] = {"ww": ww, "gb": gb, "wout": np.ascontiguousarray(wout_half)}

    in_maps = []
    for core in range(8):
        d = "fwd" if core < 4 else "rev"
        s = core % 4
        rows = slice(s * BS, (s + 1) * BS)
        im = {}
        for m in MODS:
            xm = xt[m][:, rows]
            if d == "rev":
                xm = xm[::-1]
            im[f"x_{m}"] = _bf16(xm)
            im[f"ww_{m}"] = dirs[d]["ww"][m]
            im[f"gb_{m}"] = dirs[d]["gb"][m]
        for m in ("resnet", "c3d"):
            im[f"wt_{m}"] = wt[m]
            im[f"bp_{m}"] = bp[m]
        im["wout"] = dirs[d]["wout"]
        # blend mask: fwd cores read gathered slot1 (the rev half), rev
        # cores slot0.  col0 = m, col1 = 1-m.
        mval = 1.0 if d == "fwd" else 0.0
        im["dmask"] = np.tile(np.array([[mval, 1.0 - mval]], np.float32), (128, 1))
        in_maps.append(im)
    return in_maps, has_gate_bias


def assemble(results, inputs):
    out = np.zeros((B, NCLS), np.float32)
    for s in range(4):
        rows = slice(s * BS, (s + 1) * BS)
        out[rows] = results[s]["out_partial"] + results[4 + s]["out_partial"]
    out += np.asarray(inputs["b_out"], np.float32)[None, :]
    return out


def kernel(**inputs):
    global LAST_RESULTS
    in_maps, has_gate_bias = host_prep(inputs)
    nc = build_program(has_gate_bias)
    res = bass_utils.run_bass_kernel_spmd(
        nc, in_maps, core_ids=list(range(8)), trace=TRACE)
    LAST_RESULTS = res
    return assemble(res.results, inputs)
